# revision 32
# baseline (speedup 1.0000x reference)
"""Trainium2 Bass kernel for DIF multi-head attention (disentangled attention).

Math (per batch b):
    q   = x_q  @ Wq^T  + bq        (and k, v, qp, kp, qf, kf analogously)
    C_h = q_h @ k_h^T              (content scores, per head -> output #2)
    S_h = (C_h + qp_h@kp_h^T + qf_h@kf_h^T) / 8 + mask
    P_h = softmax(S_h, axis=-1)
    ctx = concat_h(P_h @ v_h)
    out = ctx @ Wd^T + bd          (output #1)

Sharding: pure data-parallel over batch, 1 batch element per NeuronCore
(8 cores), no collectives; host shards/gathers.

Device-side formulation (per core):
  * All activations live transposed ([feature, seq]) so every matmul's
    contraction dim sits on SBUF partitions.  Weights arrive pre-transposed
    (and o-block-major for the 6 score-side projections) from the host.
  * Projections and scores are fused per o-block (= head pair) and the
    emission is software-pipelined (scores of pair b-1 run behind the PE
    while ACT/DVE drain pair b), keeping the PE stream dense.
  * Scores are computed transposed: S^T[s_k, s_q] = K_blk @ Q^T.  The content
    map is copied out (ACT) for the attention_map output, pos+feat accumulate
    in a second PSUM bank, one DVE add + one ACT exp produce the
    softmax numerator (max-subtraction is skipped: |scores| <= ~4).
  * Softmax normalization: V carries an extra all-ones column, so the AV
    matmul emits [ctx^T ; sum_k exp] in one accumulation group; the
    reciprocal row is partition-broadcast with a tiny SBUF->SBUF DMA and
    applied with one vector multiply.
  * dtypes: content path fp32r (same bits as fp32, full-rate matmul),
    pos/feat/value/AV path bf16.  The attention_map output never touches
    bf16 data.
"""

import os

import numpy as np

B, S, H, NH, D = 8, 512, 1024, 16, 64
NCORES = 8
P = 128
OB = H // P  # 8 o-blocks == head pairs
KC = H // P  # 8 k-chunks of the H contraction
SB = S // P  # 4 s-blocks
INV_SCALE = 1.0 / float(np.sqrt(D))  # 0.125

# matmul dtype knobs: 'f32r' (fast, ~fp32), 'f32' (exact, 4x slow), 'bf16'
MAIN_MM = os.environ.get("DIF_MAIN_MM", "f32r")
PF_MM = os.environ.get("DIF_PF_MM", "bf16")  # pos/feat + value/AV path


def _mmdt(mybir, name):
    return {
        "f32r": mybir.dt.float32r,
        "f32": mybir.dt.float32,
        "bf16": mybir.dt.bfloat16,
    }[name]


def _build(use_mask: bool, reps: int = 1):
    import concourse.mybir as mybir
    import concourse.tile as tile
    from concourse import bacc

    f32 = mybir.dt.float32
    bf16 = mybir.dt.bfloat16
    main_dt = _mmdt(mybir, MAIN_MM)
    mm_io_dt = main_dt
    pf_dt = _mmdt(mybir, PF_MM)
    pf_sb_dt = bf16 if PF_MM == "bf16" else f32
    pfin_dt = pf_dt if PF_MM == "bf16" else main_dt
    Ident = mybir.ActivationFunctionType.Identity
    Exp = mybir.ActivationFunctionType.Exp
    Copy = mybir.ActivationFunctionType.Copy

    def mc(ap, dt):
        return ap if ap.dtype == dt else ap.bitcast(dt)

    nc = bacc.Bacc("TRN2", target_bir_lowering=False, debug=False,
                   num_devices=NCORES)

    # ---- DRAM I/O (per core) ----
    xq = nc.dram_tensor("xq_t", [H, S], mm_io_dt, kind="ExternalInput")
    xk = nc.dram_tensor("xk_t", [H, S], mm_io_dt, kind="ExternalInput")
    xv = nc.dram_tensor("xv_t", [H, S], pfin_dt, kind="ExternalInput")
    xp = nc.dram_tensor("xp_t", [H, S], pfin_dt, kind="ExternalInput")
    xf = nc.dram_tensor("xf_t", [H, S], pfin_dt, kind="ExternalInput")
    wnames = ["q", "k", "qp", "kp", "qf", "kf"]
    wdt_of = {"q": mm_io_dt, "k": mm_io_dt, "qp": pfin_dt, "kp": pfin_dt,
              "qf": pfin_dt, "kf": pfin_dt}
    # o-block-major weight layout: wb[b, p, c*128+m] = W^T[c*128+p, b*128+m]
    wb_dram = {n: nc.dram_tensor(f"w{n}_b", [OB, P, H], wdt_of[n],
                                 kind="ExternalInput")
               for n in wnames}
    wv_dram = nc.dram_tensor("wv_t", [H, H], pfin_dt, kind="ExternalInput")
    wd_dram = nc.dram_tensor("wd_t", [H, H], mm_io_dt, kind="ExternalInput")
    ba_dram = nc.dram_tensor("bias_a", [P, 6 * OB], f32, kind="ExternalInput")
    bv_dram = nc.dram_tensor("bias_v", [1, H], mm_io_dt, kind="ExternalInput")
    bd_dram = nc.dram_tensor("bias_d", [P, OB], f32, kind="ExternalInput")
    mask_dram = None
    if use_mask:
        mask_dram = nc.dram_tensor("mask_t", [S, S], f32, kind="ExternalInput")

    am_out = nc.dram_tensor("am_t", [NH, S, S], f32, kind="ExternalOutput")
    hid_out = nc.dram_tensor("hid_t", [H, S], f32, kind="ExternalOutput")

    with tile.TileContext(nc) as tc:
        with (
            # fp32r shares the fp32 bit layout; walrus just wants the dtype
            # tags consistent along the matmul datapath
            nc.allow_low_precision(reason="fp32r/bf16 matmul datapath"),
            tc.tile_pool(name="res", bufs=1) as res,
            tc.tile_pool(name="ctxp", bufs=1) as ctxp,
        ):
            # ---- residents ----
            xts = {}

            def load_xt(nm, dram):
                t = res.tile([P, KC, S], dram.dtype, tag=f"xt_{nm}",
                             name=f"xt_{nm}")
                dv = dram[:].rearrange("(c p) s -> p c s", p=P)
                for c in range(KC):  # per-chunk: first matmul waits 256KB
                    nc.sync.dma_start(out=t[:, c, :], in_=dv[:, c, :])
                xts[nm] = t
            v_sb = [res.tile([P, NH, D + 1], pf_sb_dt, tag=f"v{j}",
                             name=f"v{j}") for j in range(SB)]
            ba_sb = res.tile([P, 6 * OB], f32, tag="ba")
            bv_sb = res.tile([1, H], mm_io_dt, tag="bv")
            bd_sb = res.tile([P, OB], f32, tag="bd")
            ones_sb = res.tile([65, P], mm_io_dt, tag="ones")
            mask_sb = None
            if use_mask:
                mask_sb = res.tile([P, SB, S], f32, tag="mask")
                nc.sync.dma_start(
                    out=mask_sb,
                    in_=mask_dram[:].rearrange("(j p) s -> p j s", p=P))
            # memset lacks fp32r/bf16 encodings: memset f32, copy-convert
            ones_f32 = res.tile([65, P], f32, tag="ones_f32")
            nc.vector.memset(ones_f32, 1.0)
            nc.vector.tensor_copy(out=ones_sb, in_=ones_f32)
            vones_f32 = res.tile([P, NH], f32, tag="vones")
            nc.vector.memset(vones_f32, 1.0)
            for j in range(SB):
                nc.vector.tensor_copy(out=v_sb[j][:, :, D:D + 1],
                                      in_=vones_f32.unsqueeze(2))

            for rep in range(reps):  # reps>1 builds are timing variants only
                xts.clear()
                # ---------------- V projection ----------------
                with (
                    tc.tile_pool(name="wvp", bufs=4) as wvp,
                    tc.tile_pool(name="vps", bufs=4, space="PSUM") as vpsp,
                ):
                    # xv tile allocated here; chunk DMAs interleave with the
                    # wv chunk DMAs below so the first matmul waits ~256KB
                    xvt = res.tile([P, KC, S], xv.dtype, tag="xt_xv",
                                   name="xt_xv")
                    xts["xv"] = xvt
                    xv_view = xv[:].rearrange("(c p) s -> p c s", p=P)
                    for half in range(2):
                        vpsums = [vpsp.tile([P, S], f32, tag="vp",
                                            name=f"vp{half}{s}")
                                  for s in range(SB)]
                        for c in range(KC):
                            if half == 0:
                                nc.sync.dma_start(out=xvt[:, c, :],
                                                  in_=xv_view[:, c, :])
                                if c == 0:
                                    nc.sync.dma_start(out=bv_sb,
                                                      in_=bv_dram[:])
                                    nc.sync.dma_start(out=ba_sb,
                                                      in_=ba_dram[:])
                                    nc.sync.dma_start(out=bd_sb,
                                                      in_=bd_dram[:])
                            wt = wvp.tile([P, S], pfin_dt, tag="wv",
                                          name="wtv")
                            nc.sync.dma_start(
                                out=wt, in_=wv_dram[c * P:(c + 1) * P,
                                                    half * S:(half + 1) * S])
                            for sb in range(SB):
                                nc.tensor.matmul(
                                    vpsums[sb],
                                    mc(xvt[:, c, sb * P:(sb + 1) * P],
                                       pfin_dt),
                                    mc(wt, pfin_dt),
                                    start=(c == 0), stop=False)
                        for sb in range(SB):
                            nc.tensor.matmul(  # bias row closes the group
                                vpsums[sb], mc(ones_sb[0:1, :], main_dt),
                                mc(bv_sb[:, half * S:(half + 1) * S],
                                   main_dt),
                                start=False, stop=True)
                            nc.vector.tensor_copy(
                                out=v_sb[sb][:, half * 8:(half + 1) * 8, 0:D],
                                in_=vpsums[sb].rearrange("p (h e) -> p h e",
                                                         e=D))

                # -------- fused projections + scores, pipelined --------
                with (
                    tc.tile_pool(name="blk", bufs=1) as blkp,
                    tc.tile_pool(name="wbp", bufs=4) as wbp,
                    tc.tile_pool(name="scb", bufs=1) as scp,
                    tc.tile_pool(name="scps", bufs=1, space="PSUM") as sps,
                ):
                    ctx_handles = {}
                    blk_tiles = {}

                    xt_src = {"q": ("xq", xq), "k": ("xk", xk),
                              "qp": ("xp", xp), "kp": ("xp", xp),
                              "qf": ("xf", xf), "kf": ("xf", xf)}

                    def emit_proj_block(b):
                        tiles = {}
                        for pn in wnames:
                            xnm, xdram = xt_src[pn]
                            if xnm not in xts:  # lazy: interleaves the input
                                load_xt(xnm, xdram)  # stream w/ weight DMAs
                            g_dt = wdt_of[pn]
                            sb_dt = main_dt if pn in ("q", "k") else pf_sb_dt
                            pidx = wnames.index(pn)
                            wt = wbp.tile([P, H], g_dt, tag="w",
                                          name=f"w{pn}{b}")
                            nc.sync.dma_start(out=wt, in_=wb_dram[pn][b])
                            ps = sps.tile([P, S], f32, tag="pp", bufs=2,
                                          name=f"pp{pn}{b}")
                            for c in range(KC):
                                nc.tensor.matmul(
                                    ps, mc(wt[:, c * P:(c + 1) * P], g_dt),
                                    mc(xts[xnm][:, c, :], g_dt),
                                    start=(c == 0), stop=(c == KC - 1))
                            ot = blkp.tile([P, S], sb_dt, tag=f"blk_{pn}",
                                           bufs=2, name=f"blk{pn}{b}")
                            nc.scalar.activation(
                                out=ot, in_=ps, func=Ident,
                                bias=ba_sb[:, pidx * OB + b:
                                           pidx * OB + b + 1],
                                scale=1.0)
                            tiles[pn] = ot
                        blk_tiles[b] = tiles

                    def hs(t, hi, jcols=None):
                        r0 = hi * D
                        if jcols is None:
                            return t[r0:r0 + D, :]
                        return t[r0:r0 + D, jcols * P:(jcols + 1) * P]

                    def emit_scores(b):
                        tiles = blk_tiles.pop(b)
                        extiles = [[None] * SB, [None] * SB]
                        for j in range(SB):
                            ps_c, ps_pf = [None, None], [None, None]
                            ams = [None, None]
                            for hi in range(2):  # alternate PE row groups
                                ps_c[hi] = sps.tile([P, S], f32, tag="sc",
                                                    bufs=3,
                                                    name=f"psc{hi}")
                                nc.tensor.matmul(
                                    ps_c[hi],
                                    mc(hs(tiles["k"], hi, j), main_dt),
                                    mc(hs(tiles["q"], hi), main_dt),
                                    start=True, stop=True)
                            for hi in range(2):
                                h = 2 * b + hi
                                am_st = scp.tile([P, S], f32, tag="am",
                                                 bufs=4, name="am_st")
                                nc.scalar.activation(out=am_st, in_=ps_c[hi],
                                                     func=Copy)
                                nc.gpsimd.dma_start(
                                    out=am_out[h, j * P:(j + 1) * P, :],
                                    in_=am_st)
                                ams[hi] = am_st
                                ps_pf[hi] = sps.tile([P, S], f32, tag="sc",
                                                     bufs=3,
                                                     name=f"pspf{hi}")
                            for hi in range(2):
                                nc.tensor.matmul(
                                    ps_pf[hi],
                                    mc(hs(tiles["kp"], hi, j), pf_dt),
                                    mc(hs(tiles["qp"], hi), pf_dt),
                                    start=True, stop=False)
                            for hi in range(2):
                                nc.tensor.matmul(
                                    ps_pf[hi],
                                    mc(hs(tiles["kf"], hi, j), pf_dt),
                                    mc(hs(tiles["qf"], hi), pf_dt),
                                    start=False, stop=True)
                            for hi in range(2):
                                sm = scp.tile([P, S], f32, tag=f"S{hi}",
                                              bufs=2, name=f"sm{hi}")
                                nc.vector.tensor_add(sm, ams[hi], ps_pf[hi])
                                ex = scp.tile([P, S], pf_sb_dt,
                                              tag=f"E{hi}{j}", bufs=2,
                                              name=f"ex{hi}{j}")
                                if use_mask:
                                    sm2 = scp.tile([P, S], f32, tag=f"M{hi}",
                                                   bufs=2, name=f"sm2{hi}")
                                    nc.vector.scalar_tensor_tensor(
                                        out=sm2, in0=sm, scalar=INV_SCALE,
                                        in1=mask_sb[:, j, :],
                                        op0=mybir.AluOpType.mult,
                                        op1=mybir.AluOpType.add)
                                    nc.scalar.activation(out=ex, in_=sm2,
                                                         func=Exp)
                                else:
                                    nc.scalar.activation(out=ex, in_=sm,
                                                         func=Exp,
                                                         scale=INV_SCALE)
                                extiles[hi][j] = ex
                        for hi in range(2):
                            h = 2 * b + hi
                            ps_x = sps.tile([D + 1, S], f32, tag="X", bufs=2,
                                            name=f"psx{hi}")
                            for j in range(SB):
                                nc.tensor.matmul(
                                    ps_x, mc(v_sb[j][:, h, :], pf_dt),
                                    mc(extiles[hi][j], pf_dt),
                                    start=(j == 0), stop=(j == SB - 1))
                            r_sb = scp.tile([D + 1, S], main_dt, tag=f"R{hi}",
                                            bufs=2, name=f"rsb{hi}")
                            nc.vector.reciprocal(out=r_sb[D:D + 1, :],
                                                 in_=ps_x[D:D + 1, :])
                            # broadcast the reciprocal row across 64
                            # partitions: K=1 outer-product matmul (row 64
                            # feeds array row-group 2), then stage in SBUF
                            ps_bc = sps.tile([D, S], f32, tag="BCp", bufs=1,
                                             name=f"psbc{hi}")
                            nc.tensor.matmul(
                                ps_bc, mc(ones_sb[D:D + 1, 0:D], main_dt),
                                mc(r_sb[D:D + 1, :], main_dt),
                                start=True, stop=True)
                            bc_sb = scp.tile([D, S], f32, tag=f"BC{hi}",
                                             bufs=2, name=f"bcsb{hi}")
                            nc.scalar.activation(out=bc_sb, in_=ps_bc,
                                                 func=Copy)
                            ctx_t = ctxp.tile([D, S], main_dt, tag=f"ctx{h}",
                                              name=f"ctx{h}")
                            nc.vector.tensor_mul(ctx_t, ps_x[0:D, :], bc_sb)
                            ctx_handles[h] = ctx_t

                    for b in range(OB):
                        emit_proj_block(b)
                        if b > 0:
                            emit_scores(b - 1)
                    emit_scores(OB - 1)

                # ---------------- output projection ----------------
                with (
                    tc.tile_pool(name="fin", bufs=1) as finp,
                    tc.tile_pool(name="fps", bufs=8, space="PSUM") as fps,
                ):
                    for half in range(2):
                        hpsums = [fps.tile([P, S], f32, tag="fp",
                                           name=f"fp{half}{b}")
                                  for b in range(4)]
                        for c in range(NH):
                            wdt = finp.tile([D, S], mm_io_dt, tag="wd",
                                            bufs=4, name="wdt")
                            nc.sync.dma_start(
                                out=wdt,
                                in_=wd_dram[c * D:(c + 1) * D,
                                            half * S:(half + 1) * S])
                            ctx_c = ctx_handles[c]
                            for b2 in range(4):
                                nc.tensor.matmul(
                                    hpsums[b2],
                                    mc(wdt[:, b2 * P:(b2 + 1) * P], main_dt),
                                    mc(ctx_c, main_dt),
                                    start=(c == 0), stop=(c == NH - 1))
                        for b2 in range(4):
                            ob = half * 4 + b2
                            hid_st = finp.tile([P, S], f32, tag="hs", bufs=4,
                                               name="hid_st")
                            nc.scalar.activation(out=hid_st, in_=hpsums[b2],
                                                 func=Ident,
                                                 bias=bd_sb[:, ob:ob + 1],
                                                 scale=1.0)
                            nc.gpsimd.dma_start(
                                out=hid_out[ob * P:(ob + 1) * P, :],
                                in_=hid_st)

    nc.compile()
    return nc


_NC_CACHE = {}


def _get_nc(use_mask: bool, reps: int = 1):
    key = (use_mask, MAIN_MM, PF_MM, reps)
    if key not in _NC_CACHE:
        _NC_CACHE[key] = _build(use_mask, reps)
    return _NC_CACHE[key]


def prep_in_maps(inputs, use_mask):
    if PF_MM == "bf16":
        import ml_dtypes
        pf_np = ml_dtypes.bfloat16
    else:
        pf_np = np.float32

    def t(a, dt=np.float32):
        return np.ascontiguousarray(
            np.asarray(a, dtype=np.float32).T.astype(dt))

    def blockify(a, dt):
        # W^T [h, o] -> [OB, P, KC*128] with wb[b, p, c*128+m] = WT[c*128+p,
        # b*128+m]; gives one contiguous per-partition run per o-block DMA
        wt = np.asarray(a, np.float32).T
        wb = wt.reshape(KC, P, OB, P).transpose(2, 1, 0, 3)
        return np.ascontiguousarray(wb.reshape(OB, P, H).astype(dt))

    WB = {n: blockify(inputs["W" + n],
                      pf_np if n in ("qp", "kp", "qf", "kf") else np.float32)
          for n in ("q", "k", "qp", "kp", "qf", "kf")}
    wv = t(inputs["Wv"], pf_np)
    wd = t(inputs["Wd"])
    ba = np.concatenate(
        [np.asarray(inputs["b" + n], np.float32).reshape(OB, P).T
         for n in ("q", "k", "qp", "kp", "qf", "kf")], axis=1)
    ba = np.ascontiguousarray(ba)
    bvv = np.ascontiguousarray(
        np.asarray(inputs["bv"], np.float32).reshape(1, H))
    bdd = np.ascontiguousarray(
        np.asarray(inputs["bd"], np.float32).reshape(OB, P).T)

    in_maps = []
    for b in range(B):
        m = {
            "xq_t": t(inputs["query_states"][b]),
            "xk_t": t(inputs["key_states"][b]),
            "xv_t": t(inputs["value_states"][b], pf_np),
            "xp_t": t(inputs["position_embedding"][b], pf_np),
            "xf_t": t(inputs["feat_embedding"][b], pf_np),
            "wv_t": wv, "wd_t": wd,
            "bias_a": ba, "bias_v": bvv, "bias_d": bdd,
        }
        for n in ("q", "k", "qp", "kp", "qf", "kf"):
            m[f"w{n}_b"] = WB[n]
        if use_mask:
            m["mask_t"] = t(inputs["attention_mask"][b, 0])
        in_maps.append(m)
    return in_maps


def kernel(query_states, key_states, value_states, attention_mask,
           position_embedding, feat_embedding,
           Wq, bq, Wk, bk, Wv, bv, Wqp, bqp, Wkp, bkp,
           Wqf, bqf, Wkf, bkf, Wd, bd):
    if "JAX_PLATFORMS" in os.environ and \
            "axon" not in os.environ["JAX_PLATFORMS"]:
        os.environ["JAX_PLATFORMS"] = "axon"

    from concourse.bass_utils import run_bass_kernel_spmd

    use_mask = bool(np.any(np.asarray(attention_mask)))
    nc = _get_nc(use_mask)
    in_maps = prep_in_maps(
        dict(query_states=query_states, key_states=key_states,
             value_states=value_states, attention_mask=attention_mask,
             position_embedding=position_embedding,
             feat_embedding=feat_embedding,
             Wq=Wq, bq=bq, Wk=Wk, bk=bk, Wv=Wv, bv=bv, Wqp=Wqp, bqp=bqp,
             Wkp=Wkp, bkp=bkp, Wqf=Wqf, bqf=bqf, Wkf=Wkf, bkf=bkf,
             Wd=Wd, bd=bd), use_mask)

    trace = bool(int(os.environ.get("DIF_TRACE", "0")))
    res = run_bass_kernel_spmd(nc, in_maps, list(range(NCORES)), trace=trace)
    global LAST_RESULT
    LAST_RESULT = res
    if trace and res.exec_time_ns is not None:
        print(f"HW exec time: {res.exec_time_ns} ns")

    hidden = np.stack([res.results[b]["hid_t"].T for b in range(B)])
    am = np.stack([res.results[b]["am_t"].transpose(0, 2, 1)
                   for b in range(B)])
    return np.ascontiguousarray(hidden), np.ascontiguousarray(am)


# revision 34
# speedup vs baseline: 1.2228x; 1.2228x over previous
"""Trainium2 Bass kernel for DIF multi-head attention (disentangled attention).

Math (per batch b):
    q   = x_q  @ Wq^T  + bq        (and k, v, qp, kp, qf, kf analogously)
    C_h = q_h @ k_h^T              (content scores, per head -> output #2)
    S_h = (C_h + qp_h@kp_h^T + qf_h@kf_h^T) / 8 + mask
    P_h = softmax(S_h, axis=-1)
    ctx = concat_h(P_h @ v_h)
    out = ctx @ Wd^T + bd          (output #1)

Sharding: pure data-parallel over batch, 1 batch element per NeuronCore
(8 cores), no collectives; host shards/gathers.

Device-side formulation (per core):
  * All activations live transposed ([feature, seq]) so every matmul's
    contraction dim sits on SBUF partitions.  Weights arrive pre-transposed
    (and o-block-major for the 6 score-side projections) from the host.
  * Projections and scores are fused per o-block (= head pair) and the
    emission is software-pipelined (scores of pair b-1 run behind the PE
    while ACT/DVE drain pair b), keeping the PE stream dense.
  * Scores are computed transposed: S^T[s_k, s_q] = K_blk @ Q^T.  The content
    map is copied out (ACT) for the attention_map output, pos+feat accumulate
    in a second PSUM bank, one DVE add + one ACT exp produce the
    softmax numerator (max-subtraction is skipped: |scores| <= ~4).
  * Softmax normalization: V carries an extra all-ones column, so the AV
    matmul emits [ctx^T ; sum_k exp] in one accumulation group; the
    reciprocal row is partition-broadcast with a tiny SBUF->SBUF DMA and
    applied with one vector multiply.
  * dtypes: content path fp32r (same bits as fp32, full-rate matmul),
    pos/feat/value/AV path bf16.  The attention_map output never touches
    bf16 data.
"""

import os

import numpy as np

B, S, H, NH, D = 8, 512, 1024, 16, 64
NCORES = 8
P = 128
OB = H // P  # 8 o-blocks == head pairs
KC = H // P  # 8 k-chunks of the H contraction
SB = S // P  # 4 s-blocks
INV_SCALE = 1.0 / float(np.sqrt(D))  # 0.125

# matmul dtype knobs: 'f32r' (fast, ~fp32), 'f32' (exact, 4x slow), 'bf16'
MAIN_MM = os.environ.get("DIF_MAIN_MM", "f32r")
PF_MM = os.environ.get("DIF_PF_MM", "bf16")  # pos/feat + value/AV path


def _mmdt(mybir, name):
    return {
        "f32r": mybir.dt.float32r,
        "f32": mybir.dt.float32,
        "bf16": mybir.dt.bfloat16,
    }[name]


def _build(use_mask: bool, reps: int = 1):
    import concourse.mybir as mybir
    import concourse.tile as tile
    from concourse import bacc

    f32 = mybir.dt.float32
    bf16 = mybir.dt.bfloat16
    main_dt = _mmdt(mybir, MAIN_MM)
    mm_io_dt = main_dt
    pf_dt = _mmdt(mybir, PF_MM)
    pf_sb_dt = bf16 if PF_MM == "bf16" else f32
    pfin_dt = pf_dt if PF_MM == "bf16" else main_dt
    Ident = mybir.ActivationFunctionType.Identity
    Exp = mybir.ActivationFunctionType.Exp
    Copy = mybir.ActivationFunctionType.Copy

    def mc(ap, dt):
        return ap if ap.dtype == dt else ap.bitcast(dt)

    nc = bacc.Bacc("TRN2", target_bir_lowering=False, debug=False,
                   num_devices=NCORES)

    # ---- DRAM I/O (per core) ----
    xq = nc.dram_tensor("xq_t", [H, S], mm_io_dt, kind="ExternalInput")
    xk = nc.dram_tensor("xk_t", [H, S], mm_io_dt, kind="ExternalInput")
    xv = nc.dram_tensor("xv_t", [H, S], pfin_dt, kind="ExternalInput")
    xp = nc.dram_tensor("xp_t", [H, S], pfin_dt, kind="ExternalInput")
    xf = nc.dram_tensor("xf_t", [H, S], pfin_dt, kind="ExternalInput")
    wnames = ["q", "k", "qp", "kp", "qf", "kf"]
    wdt_of = {"q": mm_io_dt, "k": mm_io_dt, "qp": pfin_dt, "kp": pfin_dt,
              "qf": pfin_dt, "kf": pfin_dt}
    # o-block-major weight layout: wb[b, p, c*128+m] = W^T[c*128+p, b*128+m]
    wb_dram = {n: nc.dram_tensor(f"w{n}_b", [OB, P, H], wdt_of[n],
                                 kind="ExternalInput")
               for n in wnames}
    wv_dram = nc.dram_tensor("wv_t", [H, H], pfin_dt, kind="ExternalInput")
    wd_dram = nc.dram_tensor("wd_t", [H, H], mm_io_dt, kind="ExternalInput")
    ba_dram = nc.dram_tensor("bias_a", [P, 6 * OB], f32, kind="ExternalInput")
    bv_dram = nc.dram_tensor("bias_v", [1, H], mm_io_dt, kind="ExternalInput")
    bd_dram = nc.dram_tensor("bias_d", [P, OB], f32, kind="ExternalInput")
    mask_dram = None
    if use_mask:
        mask_dram = nc.dram_tensor("mask_t", [S, S], f32, kind="ExternalInput")

    am_out = nc.dram_tensor("am_t", [NH, S, S], f32, kind="ExternalOutput")
    hid_out = nc.dram_tensor("hid_t", [H, S], f32, kind="ExternalOutput")

    with tile.TileContext(nc) as tc:
        with (
            # fp32r shares the fp32 bit layout; walrus just wants the dtype
            # tags consistent along the matmul datapath
            nc.allow_low_precision(reason="fp32r/bf16 matmul datapath"),
            tc.tile_pool(name="res", bufs=1) as res,
            tc.tile_pool(name="ctxp", bufs=1) as ctxp,
        ):
            # ---- residents ----
            xts = {}

            def load_xt(nm, dram):
                t = res.tile([P, KC, S], dram.dtype, tag=f"xt_{nm}",
                             name=f"xt_{nm}")
                dv = dram[:].rearrange("(c p) s -> p c s", p=P)
                for c in range(KC):  # per-chunk: first matmul waits 256KB
                    nc.sync.dma_start(out=t[:, c, :], in_=dv[:, c, :])
                xts[nm] = t
            v_sb = [res.tile([P, NH, D + 1], pf_sb_dt, tag=f"v{j}",
                             name=f"v{j}") for j in range(SB)]
            ba_sb = res.tile([P, 6 * OB], f32, tag="ba")
            bv_sb = res.tile([1, H], mm_io_dt, tag="bv")
            bd_sb = res.tile([P, OB], f32, tag="bd")
            ones_sb = res.tile([65, P], mm_io_dt, tag="ones")
            mask_sb = None
            if use_mask:
                mask_sb = res.tile([P, SB, S], f32, tag="mask")
                nc.sync.dma_start(
                    out=mask_sb,
                    in_=mask_dram[:].rearrange("(j p) s -> p j s", p=P))
            # memset lacks fp32r/bf16 encodings: memset f32, copy-convert
            ones_f32 = res.tile([65, P], f32, tag="ones_f32")
            nc.vector.memset(ones_f32, 1.0)
            nc.vector.tensor_copy(out=ones_sb, in_=ones_f32)
            vones_f32 = res.tile([P, NH], f32, tag="vones")
            nc.vector.memset(vones_f32, 1.0)
            for j in range(SB):
                nc.vector.tensor_copy(out=v_sb[j][:, :, D:D + 1],
                                      in_=vones_f32.unsqueeze(2))

            for rep in range(reps):  # reps>1 builds are timing variants only
                xts.clear()
                # ---------------- V projection ----------------
                with (
                    tc.tile_pool(name="wvp", bufs=4) as wvp,
                    tc.tile_pool(name="vps", bufs=4, space="PSUM") as vpsp,
                ):
                    # xv tile allocated here; chunk DMAs interleave with the
                    # wv chunk DMAs below so the first matmul waits ~256KB
                    xvt = res.tile([P, KC, S], xv.dtype, tag="xt_xv",
                                   name="xt_xv")
                    xts["xv"] = xvt
                    xv_view = xv[:].rearrange("(c p) s -> p c s", p=P)
                    for half in range(2):
                        vpsums = [vpsp.tile([P, S], f32, tag="vp",
                                            name=f"vp{half}{s}")
                                  for s in range(SB)]
                        for c in range(KC):
                            if half == 0:
                                nc.sync.dma_start(out=xvt[:, c, :],
                                                  in_=xv_view[:, c, :])
                                if c == 0:
                                    nc.sync.dma_start(out=bv_sb,
                                                      in_=bv_dram[:])
                                    nc.sync.dma_start(out=ba_sb,
                                                      in_=ba_dram[:])
                                    nc.sync.dma_start(out=bd_sb,
                                                      in_=bd_dram[:])
                            wt = wvp.tile([P, S], pfin_dt, tag="wv",
                                          name="wtv")
                            nc.sync.dma_start(
                                out=wt, in_=wv_dram[c * P:(c + 1) * P,
                                                    half * S:(half + 1) * S])
                            for sb in range(SB):
                                nc.tensor.matmul(
                                    vpsums[sb],
                                    mc(xvt[:, c, sb * P:(sb + 1) * P],
                                       pfin_dt),
                                    mc(wt, pfin_dt),
                                    start=(c == 0), stop=False)
                        for sb in range(SB):
                            nc.tensor.matmul(  # bias row closes the group
                                vpsums[sb], mc(ones_sb[0:1, :], main_dt),
                                mc(bv_sb[:, half * S:(half + 1) * S],
                                   main_dt),
                                start=False, stop=True)
                            nc.vector.tensor_copy(
                                out=v_sb[sb][:, half * 8:(half + 1) * 8, 0:D],
                                in_=vpsums[sb].rearrange("p (h e) -> p h e",
                                                         e=D))

                # -------- fused projections + scores, pipelined --------
                with (
                    tc.tile_pool(name="blk", bufs=1) as blkp,
                    tc.tile_pool(name="wbp", bufs=4) as wbp,
                    tc.tile_pool(name="scb", bufs=1) as scp,
                    tc.tile_pool(name="scps", bufs=1, space="PSUM") as sps,
                ):
                    ctx_handles = {}
                    blk_tiles = {}

                    xt_src = {"q": ("xq", xq), "k": ("xk", xk),
                              "qp": ("xp", xp), "kp": ("xp", xp),
                              "qf": ("xf", xf), "kf": ("xf", xf)}

                    def emit_proj_block(b):
                        tiles = {}
                        for pn in wnames:
                            xnm, xdram = xt_src[pn]
                            if xnm not in xts:  # lazy: interleaves the input
                                load_xt(xnm, xdram)  # stream w/ weight DMAs
                            g_dt = wdt_of[pn]
                            sb_dt = main_dt if pn in ("q", "k") else pf_sb_dt
                            pidx = wnames.index(pn)
                            wt = wbp.tile([P, H], g_dt, tag="w",
                                          name=f"w{pn}{b}")
                            nc.sync.dma_start(out=wt, in_=wb_dram[pn][b])
                            ps = sps.tile([P, S], f32, tag="pp", bufs=2,
                                          name=f"pp{pn}{b}")
                            for c in range(KC):
                                nc.tensor.matmul(
                                    ps, mc(wt[:, c * P:(c + 1) * P], g_dt),
                                    mc(xts[xnm][:, c, :], g_dt),
                                    start=(c == 0), stop=(c == KC - 1))
                            ot = blkp.tile([P, S], sb_dt, tag=f"blk_{pn}",
                                           bufs=2, name=f"blk{pn}{b}")
                            nc.scalar.activation(
                                out=ot, in_=ps, func=Ident,
                                bias=ba_sb[:, pidx * OB + b:
                                           pidx * OB + b + 1],
                                scale=1.0)
                            tiles[pn] = ot
                        blk_tiles[b] = tiles

                    def hs(t, hi, jcols=None):
                        r0 = hi * D
                        if jcols is None:
                            return t[r0:r0 + D, :]
                        return t[r0:r0 + D, jcols * P:(jcols + 1) * P]

                    def emit_scores(b):
                        tiles = blk_tiles.pop(b)
                        extiles = [[None] * SB, [None] * SB]
                        for j in range(SB):
                            ps_c, ps_pf = [None, None], [None, None]
                            ams = [None, None]
                            for hi in range(2):  # alternate PE row groups
                                ps_c[hi] = sps.tile([P, S], f32, tag="sc",
                                                    bufs=3,
                                                    name=f"psc{hi}")
                                nc.tensor.matmul(
                                    ps_c[hi],
                                    mc(hs(tiles["k"], hi, j), main_dt),
                                    mc(hs(tiles["q"], hi), main_dt),
                                    start=True, stop=True)
                            for hi in range(2):
                                h = 2 * b + hi
                                am_st = scp.tile([P, S], f32, tag="am",
                                                 bufs=4, name="am_st")
                                nc.scalar.activation(out=am_st, in_=ps_c[hi],
                                                     func=Copy)
                                nc.gpsimd.dma_start(
                                    out=am_out[h, j * P:(j + 1) * P, :],
                                    in_=am_st)
                                ams[hi] = am_st
                                ps_pf[hi] = sps.tile([P, S], f32, tag="sc",
                                                     bufs=3,
                                                     name=f"pspf{hi}")
                            for hi in range(2):
                                nc.tensor.matmul(
                                    ps_pf[hi],
                                    mc(hs(tiles["kp"], hi, j), pf_dt),
                                    mc(hs(tiles["qp"], hi), pf_dt),
                                    start=True, stop=False)
                            for hi in range(2):
                                nc.tensor.matmul(
                                    ps_pf[hi],
                                    mc(hs(tiles["kf"], hi, j), pf_dt),
                                    mc(hs(tiles["qf"], hi), pf_dt),
                                    start=False, stop=True)
                            for hi in range(2):
                                sm = scp.tile([P, S], f32, tag=f"S{hi}",
                                              bufs=2, name=f"sm{hi}")
                                nc.vector.tensor_add(sm, ams[hi], ps_pf[hi])
                                ex = scp.tile([P, S], pf_sb_dt,
                                              tag=f"E{hi}{j}", bufs=2,
                                              name=f"ex{hi}{j}")
                                if use_mask:
                                    sm2 = scp.tile([P, S], f32, tag=f"M{hi}",
                                                   bufs=2, name=f"sm2{hi}")
                                    nc.vector.scalar_tensor_tensor(
                                        out=sm2, in0=sm, scalar=INV_SCALE,
                                        in1=mask_sb[:, j, :],
                                        op0=mybir.AluOpType.mult,
                                        op1=mybir.AluOpType.add)
                                    nc.scalar.activation(out=ex, in_=sm2,
                                                         func=Exp)
                                else:
                                    nc.scalar.activation(out=ex, in_=sm,
                                                         func=Exp,
                                                         scale=INV_SCALE)
                                extiles[hi][j] = ex
                        for hi in range(2):
                            h = 2 * b + hi
                            ps_x = sps.tile([D + 1, S], f32, tag="X", bufs=2,
                                            name=f"psx{hi}")
                            for j in range(SB):
                                nc.tensor.matmul(
                                    ps_x, mc(v_sb[j][:, h, :], pf_dt),
                                    mc(extiles[hi][j], pf_dt),
                                    start=(j == 0), stop=(j == SB - 1))
                            r_sb = scp.tile([D + 1, S], main_dt, tag=f"R{hi}",
                                            bufs=2, name=f"rsb{hi}")
                            nc.vector.reciprocal(out=r_sb[D:D + 1, :],
                                                 in_=ps_x[D:D + 1, :])
                            # broadcast the reciprocal row across 64
                            # partitions: K=1 outer-product matmul (row 64
                            # feeds array row-group 2), then stage in SBUF
                            ps_bc = sps.tile([D, S], f32, tag="BCp", bufs=1,
                                             name=f"psbc{hi}")
                            nc.tensor.matmul(
                                ps_bc, mc(ones_sb[D:D + 1, 0:D], main_dt),
                                mc(r_sb[D:D + 1, :], main_dt),
                                start=True, stop=True)
                            bc_sb = scp.tile([D, S], f32, tag=f"BC{hi}",
                                             bufs=2, name=f"bcsb{hi}")
                            nc.scalar.activation(out=bc_sb, in_=ps_bc,
                                                 func=Copy)
                            ctx_t = ctxp.tile([D, S], main_dt, tag=f"ctx{h}",
                                              name=f"ctx{h}")
                            nc.vector.tensor_mul(ctx_t, ps_x[0:D, :], bc_sb)
                            ctx_handles[h] = ctx_t

                    for b in range(OB):
                        emit_proj_block(b)
                        if b > 0:
                            emit_scores(b - 1)
                    emit_scores(OB - 1)

                # ---------------- output projection ----------------
                with (
                    tc.tile_pool(name="fin", bufs=1) as finp,
                    tc.tile_pool(name="fps", bufs=8, space="PSUM") as fps,
                ):
                    for half in range(2):
                        hpsums = [fps.tile([P, S], f32, tag="fp",
                                           name=f"fp{half}{b}")
                                  for b in range(4)]
                        for c in range(NH):
                            wdt = finp.tile([D, S], mm_io_dt, tag="wd",
                                            bufs=4, name="wdt")
                            nc.sync.dma_start(
                                out=wdt,
                                in_=wd_dram[c * D:(c + 1) * D,
                                            half * S:(half + 1) * S])
                            ctx_c = ctx_handles[c]
                            for b2 in range(4):
                                nc.tensor.matmul(
                                    hpsums[b2],
                                    mc(wdt[:, b2 * P:(b2 + 1) * P], main_dt),
                                    mc(ctx_c, main_dt),
                                    start=(c == 0), stop=(c == NH - 1))
                        for b2 in range(4):
                            ob = half * 4 + b2
                            hid_st = finp.tile([P, S], f32, tag="hs", bufs=4,
                                               name="hid_st")
                            nc.scalar.activation(out=hid_st, in_=hpsums[b2],
                                                 func=Ident,
                                                 bias=bd_sb[:, ob:ob + 1],
                                                 scale=1.0)
                            nc.gpsimd.dma_start(
                                out=hid_out[ob * P:(ob + 1) * P, :],
                                in_=hid_st)

    nc.compile()
    return nc


_NC_CACHE = {}


def _get_nc(use_mask: bool, reps: int = 1):
    key = (use_mask, MAIN_MM, PF_MM, reps)
    if key not in _NC_CACHE:
        _NC_CACHE[key] = _build(use_mask, reps)
    return _NC_CACHE[key]


def prep_in_maps(inputs, use_mask):
    if PF_MM == "bf16":
        import ml_dtypes
        pf_np = ml_dtypes.bfloat16
    else:
        pf_np = np.float32
    if MAIN_MM == "bf16":
        import ml_dtypes
        main_np = ml_dtypes.bfloat16
    else:
        main_np = np.float32

    def t(a, dt=np.float32):
        return np.ascontiguousarray(
            np.asarray(a, dtype=np.float32).T.astype(dt))

    def blockify(a, dt):
        # W^T [h, o] -> [OB, P, KC*128] with wb[b, p, c*128+m] = WT[c*128+p,
        # b*128+m]; gives one contiguous per-partition run per o-block DMA
        wt = np.asarray(a, np.float32).T
        wb = wt.reshape(KC, P, OB, P).transpose(2, 1, 0, 3)
        return np.ascontiguousarray(wb.reshape(OB, P, H).astype(dt))

    WB = {n: blockify(inputs["W" + n],
                      pf_np if n in ("qp", "kp", "qf", "kf") else main_np)
          for n in ("q", "k", "qp", "kp", "qf", "kf")}
    wv = t(inputs["Wv"], pf_np)
    wd = t(inputs["Wd"], main_np)
    ba = np.concatenate(
        [np.asarray(inputs["b" + n], np.float32).reshape(OB, P).T
         for n in ("q", "k", "qp", "kp", "qf", "kf")], axis=1)
    ba = np.ascontiguousarray(ba)
    bvv = np.ascontiguousarray(
        np.asarray(inputs["bv"], np.float32).reshape(1, H).astype(main_np))
    bdd = np.ascontiguousarray(
        np.asarray(inputs["bd"], np.float32).reshape(OB, P).T)

    in_maps = []
    for b in range(B):
        m = {
            "xq_t": t(inputs["query_states"][b], main_np),
            "xk_t": t(inputs["key_states"][b], main_np),
            "xv_t": t(inputs["value_states"][b], pf_np),
            "xp_t": t(inputs["position_embedding"][b], pf_np),
            "xf_t": t(inputs["feat_embedding"][b], pf_np),
            "wv_t": wv, "wd_t": wd,
            "bias_a": ba, "bias_v": bvv, "bias_d": bdd,
        }
        for n in ("q", "k", "qp", "kp", "qf", "kf"):
            m[f"w{n}_b"] = WB[n]
        if use_mask:
            m["mask_t"] = t(inputs["attention_mask"][b, 0])
        in_maps.append(m)
    return in_maps


def kernel(query_states, key_states, value_states, attention_mask,
           position_embedding, feat_embedding,
           Wq, bq, Wk, bk, Wv, bv, Wqp, bqp, Wkp, bkp,
           Wqf, bqf, Wkf, bkf, Wd, bd):
    if "JAX_PLATFORMS" in os.environ and \
            "axon" not in os.environ["JAX_PLATFORMS"]:
        os.environ["JAX_PLATFORMS"] = "axon"

    from concourse.bass_utils import run_bass_kernel_spmd

    use_mask = bool(np.any(np.asarray(attention_mask)))
    nc = _get_nc(use_mask)
    in_maps = prep_in_maps(
        dict(query_states=query_states, key_states=key_states,
             value_states=value_states, attention_mask=attention_mask,
             position_embedding=position_embedding,
             feat_embedding=feat_embedding,
             Wq=Wq, bq=bq, Wk=Wk, bk=bk, Wv=Wv, bv=bv, Wqp=Wqp, bqp=bqp,
             Wkp=Wkp, bkp=bkp, Wqf=Wqf, bqf=bqf, Wkf=Wkf, bkf=bkf,
             Wd=Wd, bd=bd), use_mask)

    trace = bool(int(os.environ.get("DIF_TRACE", "0")))
    res = run_bass_kernel_spmd(nc, in_maps, list(range(NCORES)), trace=trace)
    global LAST_RESULT
    LAST_RESULT = res
    if trace and res.exec_time_ns is not None:
        print(f"HW exec time: {res.exec_time_ns} ns")

    hidden = np.stack([res.results[b]["hid_t"].T for b in range(B)])
    am = np.stack([res.results[b]["am_t"].transpose(0, 2, 1)
                   for b in range(B)])
    return np.ascontiguousarray(hidden), np.ascontiguousarray(am)
